# revision 29
# baseline (speedup 1.0000x reference)
"""Multi-head attention (B=2, L=2048, D=1024, H=16) on 8 trn2 NeuronCores.

Sharding: core c = (batch b = c // 4, head-group g = c % 4); each group owns 4
heads (256 dims). Q/K/V projections are column-parallel per group, attention is
fully local per (batch, head), fc is row-parallel with the 4 group partials of
each batch summed on the host.

Per-core dataflow (matmul operands bf16, PSUM accumulation fp32):
  qT,kT [256, L] = W @ x.T          (host supplies x.T and W.T slices)
  v     [L, 256] (+ ones column)    (bias via augmented contraction row)

Attention runs in (key-block, head-pair) units. The two heads of a pair live
in partitions 0-63 / 64-127 of one kt/qt tile, so their score matmuls are
64x128 row-tiled (tile_position (0,0) and (64,0)) and execute CONCURRENTLY on
the PE when emitted back to back into different PSUM banks of one shared
[128, 1024] tile. One ScalarE exp covers both heads; masked diagonal columns
are computed as real scores (finite) and zeroed by gpsimd mask multiplies, so
the merged exp never sees stale PSUM. PV keeps the augmented-v form (M=65,
row 64 = softmax denominator). Normalization: DVE reciprocal straight from
PSUM row 64 -> gpsimd partition_broadcast -> DVE multiply into ctx.

Critical path: only the kb0-3 K quarter and the span-0 Q quarter are
projected before attention starts, so the first exp fires ~20us in (vs ~41us
when K/Q are fully projected first). All other projections (K rest, V, Q
half 1) and the fc units run as cost-paced PE fillers inside the exp-chain
shadow. DMA triggers are split across the sync/scalar/gpsimd queues so the
critical K and Q tiles race down independent queues at t=0.
"""

import numpy as np
import ml_dtypes

import concourse.bass as bass
import concourse.mybir as mybir
import concourse.tile as tile
from concourse import bacc, bass_utils

L = 2048
D = 1024
DK = 64
GH = 4            # heads per core
DG = 256          # dims per core
NB = L // 128     # 16 key/query blocks
NSPAN = L // 512  # 4 query spans
F32 = mybir.dt.float32
BF = mybir.dt.bfloat16
U8 = mybir.dt.uint8

_CACHE: dict = {}
LAST_EXEC_NS = None
TRACE = False


def _install_ntff_hook():
    """Register the axon NTFF profiling hook that this image's antenv lacks."""
    import contextlib
    import ctypes
    import sys
    import types

    try:
        from antenv.axon_hooks import get_axon_ntff_profile_hook  # noqa: F401
        return
    except ImportError:
        pass
    import antenv

    mod = types.ModuleType("antenv.axon_hooks")
    state = {"hook": None}
    mod.set_axon_ntff_profile_hook = lambda h: state.__setitem__("hook", h)
    mod.get_axon_ntff_profile_hook = lambda: state["hook"]
    sys.modules["antenv.axon_hooks"] = mod
    antenv.axon_hooks = mod

    so_path = "/opt/axon/libaxon_pjrt.so"
    lib = ctypes.CDLL(so_path)
    if not hasattr(lib, "axon_start_nrt_profile"):
        return
    lib.axon_start_nrt_profile.argtypes = [
        ctypes.POINTER(ctypes.c_int64),
        ctypes.c_size_t,
    ]
    lib.axon_start_nrt_profile.restype = ctypes.c_int64
    lib.axon_stop_nrt_profile.argtypes = [ctypes.c_char_p]
    lib.axon_stop_nrt_profile.restype = ctypes.c_int64

    @contextlib.contextmanager
    def _hook(output_dir, device_ids):
        import jax

        jax.devices()
        if device_ids:
            ids = (ctypes.c_int64 * len(device_ids))(*device_ids)
            rc = lib.axon_start_nrt_profile(ids, len(device_ids))
        else:
            rc = lib.axon_start_nrt_profile(None, 0)
        if rc != 0:
            raise RuntimeError(f"axon_start_nrt_profile rc={rc}")
        try:
            yield
        finally:
            n = lib.axon_stop_nrt_profile(str(output_dir).encode())
            print(f"profile: {n} file(s) written to {output_dir}", file=sys.stderr)

    state["hook"] = _hook


def _classify(mask2d: np.ndarray) -> np.ndarray:
    """cls[qb, kb]: 0 = all masked (dead), 1 = all unmasked (pure), 2 = mixed."""
    m = mask2d.astype(np.uint8).reshape(NB, 128, NB, 128)
    s = m.sum(axis=(1, 3))
    cls = np.full((NB, NB), 2, np.int8)
    cls[s == 0] = 0
    cls[s == 128 * 128] = 1
    return cls


def _mixed_list(cls):
    return [(qb, kb) for qb in range(NB) for kb in range(NB) if cls[qb, kb] == 2]


DEBUG_DUMP = False


def _build(cls: np.ndarray, zv: bool = False, zf: bool = False):
    nc = bacc.Bacc("TRN2", target_bir_lowering=False, debug=False, num_devices=8)
    XTQ = nc.dram_tensor("XTQ", [D, L], BF, kind="ExternalInput").ap()
    XTK = nc.dram_tensor("XTK", [D, L], BF, kind="ExternalInput").ap()
    XTV = nc.dram_tensor("XTV", [D + 1, L], BF, kind="ExternalInput").ap()
    WQT = nc.dram_tensor("WQT", [128, 8, DG], BF, kind="ExternalInput").ap()
    WKT = nc.dram_tensor("WKT", [128, 8, DG], BF, kind="ExternalInput").ap()
    WVT = nc.dram_tensor("WVT", [128, 8, DG], BF, kind="ExternalInput").ap()
    VROW = nc.dram_tensor("VROW", [1, DG], BF, kind="ExternalInput").ap()
    BQ = nc.dram_tensor("BQ", [128, 2, 1], F32, kind="ExternalInput").ap()
    BK = nc.dram_tensor("BK", [128, 2, 1], F32, kind="ExternalInput").ap()
    FCT = nc.dram_tensor("FCT", [128, 2, D], BF, kind="ExternalInput").ap()
    FCB = nc.dram_tensor("FCB", [1, D], BF, kind="ExternalInput").ap()
    mixed = _mixed_list(cls)
    nmix = max(1, len(mixed))
    MCHUNKS = nc.dram_tensor("MCHUNKS", [nmix, 128, 128], U8, kind="ExternalInput").ap()
    Y = nc.dram_tensor("Y", [L, D], BF, kind="ExternalOutput").ap()
    PTD = DEND = None
    if DEBUG_DUMP:
        PTD = nc.dram_tensor("PTD", [128, 2, 1024], BF, kind="ExternalOutput").ap()
        DEND = nc.dram_tensor("DEND", [2, 512], F32, kind="ExternalOutput").ap()

    # per-span live key blocks (shared by all heads; mask broadcasts)
    span_kbs = []
    for s in range(NSPAN):
        kbs = [kb for kb in range(NB) if any(cls[4 * s + j, kb] for j in range(4))]
        assert kbs, f"query span {s} has no unmasked keys"
        span_kbs.append(kbs)

    Exp = mybir.ActivationFunctionType.Exp

    with tile.TileContext(nc) as tc:
        with (
            tc.tile_pool(name="w", bufs=1) as wp,
            tc.tile_pool(name="xs", bufs=24) as xp,
            tc.tile_pool(name="keep", bufs=1) as kp,
            tc.tile_pool(name="ptp", bufs=5) as ptp,
            tc.tile_pool(name="sm", bufs=2) as smp,
            tc.tile_pool(name="ev", bufs=4) as evp,
            tc.tile_pool(name="pout", bufs=2, space="PSUM") as pout,
            tc.tile_pool(name="psc", bufs=2, space="PSUM") as psc,
            tc.tile_pool(name="pov", bufs=2, space="PSUM") as pov,
        ):
            # ---------------- persistent activations ----------------
            qt_sb = [kp.tile([128, L], BF, tag=f"qt{i}", name=f"qt{i}") for i in range(2)]
            kt_sb = [kp.tile([128, L], BF, tag=f"kt{i}", name=f"kt{i}") for i in range(2)]
            ctx_sb = [kp.tile([128, L], BF, tag=f"ctx{i}", name=f"ctx{i}") for i in range(2)]
            v_sb = kp.tile([128, NB, GH, 65], BF, tag="vsb")

            # ---------------- weights ----------------
            wkt0 = wp.tile([128, DG], BF, tag="wkt0", name="wkt0")
            wkt17 = wp.tile([128, 7, DG], BF, tag="wkt17", name="wkt17")
            wqt = wp.tile([128, 8, DG], BF, tag="wqt")
            wvt = wp.tile([128, 8, DG], BF, tag="wvt")
            vrow = wp.tile([1, DG], BF, tag="vrow")
            fct = wp.tile([128, 2, D], BF, tag="fct")
            fcb = wp.tile([1, D], BF, tag="fcb")
            bq = wp.tile([128, 2, 1], F32, tag="bq")
            bk = wp.tile([128, 2, 1], F32, tag="bk")

            # ---------------- DMA staging ----------------
            # everything on the sync queue (compute-engine DMA triggers
            # measured 2-4x slower), strictly in consumption order: K weights
            # first (a chain stalls on wkt17 if it queues behind the x
            # stream), then the critical K/Q quarter tiles, then the rest.
            nc.sync.dma_start(out=wkt0[:], in_=WKT[:, 0])
            nc.sync.dma_start(out=wkt17[:], in_=WKT[:, 1:8])
            nc.sync.dma_start(out=wqt[:], in_=WQT[:])

            def load_half(src, half, name, cols=None):
                # one tile + DMA per k-tile; cols=(lo, hi) loads only that
                # column slice (the other slice is DMAed later). All 48 x
                # tiles are DMA-triggered up front, so every tile gets its
                # own slot (bufs=48) - slot rotation would clobber tiles
                # whose reader chains are emitted later as fillers.
                ts = []
                lo, hi = cols or (0, 1024)
                for kt in range(8):
                    t = xp.tile([128, 1024], BF, tag="xt", bufs=48,
                                name=f"{name}{half}k{kt}")
                    nc.sync.dma_start(
                        out=t[:, lo:hi],
                        in_=src[kt * 128:(kt + 1) * 128,
                                half * 1024 + lo:half * 1024 + hi],
                    )
                    ts.append(t)
                return ts

            def load_cols(ts, src, half, lo, hi):
                for kt in range(8):
                    nc.sync.dma_start(
                        out=ts[kt][:, lo:hi],
                        in_=src[kt * 128:(kt + 1) * 128,
                                half * 1024 + lo:half * 1024 + hi],
                    )

            xtk0_ts = load_half(XTK, 0, "xtk", cols=(0, 512))
            xtq0_ts = load_half(XTQ, 0, "xtq", cols=(0, 512))
            nc.sync.dma_start(out=bk[:], in_=BK[:])
            nc.sync.dma_start(out=bq[:], in_=BQ[:])
            load_cols(xtk0_ts, XTK, 0, 512, 1024)
            load_cols(xtq0_ts, XTQ, 0, 512, 1024)

            # 0/1 chunks for mixed mask blocks (needed by span 0 already)
            m01_idx = {qk: i for i, qk in enumerate(mixed)}
            m01_all = wp.tile([128, nmix, 128], BF, tag="m01")
            mstage = wp.tile([128, nmix, 128], U8, tag="mstage")
            if mixed:
                nc.sync.dma_start(out=mstage[:], in_=MCHUNKS.rearrange("n p c -> p n c"))
                nc.gpsimd.tensor_copy(m01_all[:], mstage[:])

            # rest of the x stream, in rough consumption order
            nc.sync.dma_start(out=wvt[:], in_=WVT[:])
            nc.sync.dma_start(out=vrow[:], in_=VROW[:])
            xr = xp.tile([1, L], BF, tag="xtr", bufs=1, name="xr")
            nc.sync.dma_start(out=xr[:], in_=XTV[D:D + 1])
            xtv_ts = load_half(XTV, 0, "xtv")
            xtk1_ts = load_half(XTK, 1, "xtk")
            xtv_ts += load_half(XTV, 1, "xtv")
            xtq1_ts = load_half(XTQ, 1, "xtq")
            nc.sync.dma_start(out=fct[:], in_=FCT[:])
            nc.sync.dma_start(out=fcb[:], in_=FCB[:])
            xtk_halves = [xtk0_ts, xtk1_ts]
            xtq_halves = [xtq0_ts, xtq1_ts]

            # ---------------- constants (off the DMA-trigger path) --------
            ctx1 = wp.tile([1, L], BF, tag="ctx1")
            nc.vector.memset(ctx1[:], 1.0)
            nc.vector.memset(v_sb[:, :, :, 64:65], 1.0)

            wk_slice = lambda kt, mch: (
                wkt0[:, mch * 128:(mch + 1) * 128] if kt == 0
                else wkt17[:, kt - 1, mch * 128:(mch + 1) * 128]
            )
            wq_slice = lambda kt, mch: wqt[:, kt, mch * 128:(mch + 1) * 128]

            def proj_chain(q, mch, halves, wsl, bias, dst):
                # one L-quarter (512 cols) x one 128-row output chunk
                ts = halves[q // 2]
                c0 = (q % 2) * 512
                p = pout.tile([128, 512], F32, tag="out", name=f"pp{q}_{mch}")
                for kt in range(8):
                    nc.tensor.matmul(
                        p[:], wsl(kt, mch), ts[kt][:, c0:c0 + 512],
                        start=(kt == 0), stop=(kt == 7),
                    )
                nc.vector.tensor_scalar_add(
                    dst[mch][:, q * 512:(q + 1) * 512], p[:], bias[:, mch]
                )

            v_done = [False] * NB

            def v_chunk(lb):
                if v_done[lb]:
                    return
                v_done[lb] = True
                half, lbl = lb // 8, lb % 8
                p = pout.tile([128, DG], F32, tag="out", name=f"pv{lb}")
                for kt in range(8):
                    nc.tensor.matmul(
                        p[:],
                        xtv_ts[half * 8 + kt][:, lbl * 128:(lbl + 1) * 128],
                        wvt[:, kt],
                        start=(kt == 0),
                        stop=(zv and kt == 7),
                    )
                if not zv:
                    nc.tensor.matmul(
                        p[:], xr[:, lb * 128:(lb + 1) * 128], vrow[:],
                        start=False, stop=True,
                    )
                nc.vector.tensor_copy(
                    v_sb[:, lb, :, 0:64], p[:].rearrange("p (h d) -> p h d", h=GH)
                )

            drain_fc = [False]

            def fc_unit(lb, nh):
                ys = evp.tile([128, 512], BF, tag="ys", name=f"ys{lb}_{nh}")
                pool = psc if drain_fc[0] else pout
                yp = pool.tile([128, 512], F32, tag="sc" if pool is psc else "out",
                               name=f"yp{lb}_{nh}")
                nc.tensor.matmul(
                    yp[:], ctx_sb[0][:, lb * 128:(lb + 1) * 128],
                    fct[:, 0, nh * 512:(nh + 1) * 512], start=True, stop=False,
                    skip_group_check=True,
                )
                nc.tensor.matmul(
                    yp[:], ctx_sb[1][:, lb * 128:(lb + 1) * 128],
                    fct[:, 1, nh * 512:(nh + 1) * 512], start=False, stop=zf,
                    skip_group_check=True,
                )
                if not zf:
                    nc.tensor.matmul(
                        yp[:], ctx1[:, lb * 128:(lb + 1) * 128],
                        fcb[:, nh * 512:(nh + 1) * 512], start=False, stop=True,
                        skip_group_check=True,
                    )
                nc.vector.tensor_copy(ys[:], yp[:])
                nc.sync.dma_start(
                    out=Y[lb * 128:(lb + 1) * 128, nh * 512:(nh + 1) * 512],
                    in_=ys[:],
                )

            # ---------------- filler machinery ----------------
            # (cost_ns, key, closure); popped against a per-unit budget so
            # the PE always has work inside the exp-chain shadow without
            # starving the score stream. force_keys() hard-pops (in order)
            # until named producers have been EMITTED - emission order is
            # dependency order for the tile framework.
            fillers = []
            urgent = []
            budget = [0.0]
            done_keys = set()

            def _pop_filler():
                c, k, f = fillers.pop(0)
                budget[0] = max(budget[0] - c, -1500.0)
                if k is not None:
                    done_keys.add(k)
                f()

            def pace(amount):
                budget[0] += amount
                while fillers and fillers[0][0] <= budget[0]:
                    _pop_filler()

            def force_keys(keys):
                keys = [k for k in keys if k not in done_keys]
                while keys and fillers:
                    _pop_filler()
                    keys = [k for k in keys if k not in done_keys]
                assert not keys, f"missing filler producers {keys}"

            def flush_urgent():
                while urgent:
                    urgent.pop(0)()

            # ---------------- attention ----------------
            def first_col(kb, s):
                if kb == span_kbs[s][0]:
                    return 0
                j0 = next(j for j in range(4) if cls[4 * s + j, kb])
                return j0 * 128

            def norm_prep(st):
                # NOTE: reciprocal_approx_fast mis-reads a PSUM input with a
                # nonzero partition base (it reads partition 0) - copy the
                # denominator row to SBUF partition 0 first.
                outp = st["outp"]
                srow = smp.tile([1, 512], F32, tag="srow", name="srow")
                nc.vector.tensor_copy(srow[:], outp[64:65, :])
                rc = smp.tile([1, 512], F32, tag="rc", name="rc")
                nc.vector.reciprocal_approx_fast(rc[:], srow[:])
                bcs = smp.tile([64, 512], F32, tag="bcs", name="bcs")
                nc.gpsimd.partition_broadcast(bcs[:], rc[:])
                st["bcs"] = bcs
                if DEBUG_DUMP and st["s"] == 0 and st["h"] < 2:
                    nc.sync.dma_start(out=DEND[st["h"]:st["h"] + 1], in_=bcs[0:1, :])

            def norm_apply(st, on_done=None):
                h, s, outp, bcs = st["h"], st["s"], st["outp"], st["bcs"]
                hi, ho = h // 2, (h % 2) * 64
                nc.vector.tensor_mul(
                    ctx_sb[hi][ho:ho + 64, s * 512:(s + 1) * 512],
                    outp[0:64, :],
                    bcs[:],
                )
                if on_done is not None:
                    on_done()

            pending_pv = []

            def run_pv(n_keep):
                while len(pending_pv) > n_keep:
                    pending_pv.pop(0)()

            def emit_span(s):
                kbs = span_kbs[s]
                lag = 2 if s == 0 else 1
                for hp in range(2):
                    # the previous pair's norms MUST be emitted before this
                    # pair's PVs reuse their PSUM slots (emission order is
                    # dependency order)
                    flush_urgent()
                    outps = [
                        pov.tile([65, 512], F32, tag="ov", name=f"ov{2 * hp + i}_{s}")
                        for i in range(2)
                    ]
                    for ki, kb in enumerate(kbs):
                        c0 = first_col(kb, s)
                        scp = psc.tile([128, 1024], F32, tag="sc", name="scp")
                        # paired 64-row scores: heads 2hp / 2hp+1 in row tiles
                        # (0,0) and (64,0), concurrent on the PE
                        for hh in range(2):
                            nc.tensor.matmul(
                                scp[:, hh * 512 + c0:(hh + 1) * 512],
                                kt_sb[hp][hh * 64:(hh + 1) * 64,
                                          kb * 128:(kb + 1) * 128],
                                qt_sb[hp][hh * 64:(hh + 1) * 64,
                                          s * 512 + c0:(s + 1) * 512],
                                start=True, stop=True,
                            )
                        pt = ptp.tile([128, 1024], BF, tag="pt", name="pt")
                        # one exp for both heads; [512:512+c0] is stale PSUM
                        # whose exp lands in a pt strip nothing ever reads
                        nc.scalar.activation(pt[:, c0:], scp[:, c0:], Exp, scale=0.125)
                        for j in range(c0 // 128, 4):
                            qb = 4 * s + j
                            c = cls[qb, kb]
                            if c == 1:
                                continue
                            for hh in range(2):
                                sl = pt[:, hh * 512 + j * 128:hh * 512 + (j + 1) * 128]
                                if c == 0:
                                    nc.vector.memset(sl, 0.0)
                                else:
                                    nc.vector.tensor_mul(
                                        sl, sl, m01_all[:, m01_idx[(qb, kb)], :]
                                    )

                        def pv(kb=kb, c0=c0, pt=pt, outps=outps, hp=hp,
                               first=(ki == 0), last=(ki == len(kbs) - 1)):
                            v_chunk(kb)  # backstop; usually a no-op
                            for hh in range(2):
                                nc.tensor.matmul(
                                    outps[hh][:, c0:],
                                    v_sb[:, kb, 2 * hp + hh, :],
                                    pt[:, hh * 512 + c0:(hh + 1) * 512],
                                    start=first, stop=last,
                                    skip_group_check=True,
                                )

                        if DEBUG_DUMP and s == 0 and hp == 0 and kb <= 1:
                            nc.sync.dma_start(out=PTD[:, kb], in_=pt[:])
                        pending_pv.append(pv)
                        run_pv(lag)
                        pace(560)
                    run_pv(0)
                    for hh in range(2):
                        h = 2 * hp + hh
                        st = {"h": h, "s": s, "outp": outps[hh]}
                        on_done = None
                        if hp == 1 and hh == 1:
                            def on_done(s=s):
                                # span ctx complete -> fc is legal; front of
                                # the queue so fc drains early (small units,
                                # latency-critical at the end of the kernel)
                                for i, lb in enumerate(range(4 * s, 4 * s + 4)):
                                    for nh in range(2):
                                        fillers.insert(
                                            2 * i + nh,
                                            (430, None,
                                             lambda lb=lb, nh=nh: fc_unit(lb, nh)))
                        urgent.append(lambda st=st: norm_prep(st))
                        urgent.append(lambda st=st, od=on_done: norm_apply(st, od))

            # critical-path projections: K quarter 0 (kb 0-3) + Q span 0
            for mch in range(2):
                proj_chain(0, mch, xtk_halves, wk_slice, bk, kt_sb)
            for mch in range(2):
                proj_chain(0, mch, xtq_halves, wq_slice, bq, qt_sb)

            # everything else becomes fillers, ordered by first use
            CH = 1700

            def add_k(q):
                for mch in range(2):
                    fillers.append((CH, ("k", q, mch), lambda q=q, mch=mch: proj_chain(
                        q, mch, xtk_halves, wk_slice, bk, kt_sb)))

            def add_q(q):
                for mch in range(2):
                    fillers.append((CH, ("q", q, mch), lambda q=q, mch=mch: proj_chain(
                        q, mch, xtq_halves, wq_slice, bq, qt_sb)))

            def add_v(lbs):
                for lb in lbs:
                    fillers.append((870, None, lambda lb=lb: v_chunk(lb)))

            add_k(1)
            add_q(1)
            add_v(range(4))
            add_k(2)
            add_q(2)
            add_k(3)
            add_q(3)
            add_v(range(4, 16))

            def span_need(s):
                ks = {("k", q, mch) for q in range(1, s + 1) for mch in range(2)}
                return ks | {("q", s, mch) for mch in range(2)}

            for s in range(NSPAN):
                if s:
                    force_keys(span_need(s))
                emit_span(s)
            run_pv(0)
            flush_urgent()
            drain_fc[0] = True
            while fillers:
                fillers.pop(0)[2]()

            if DEBUG_DUMP:
                QTD = nc.dram_tensor("QTD", [128, 2, L], BF, kind="ExternalOutput").ap()
                KTD = nc.dram_tensor("KTD", [128, 2, L], BF, kind="ExternalOutput").ap()
                CTXD = nc.dram_tensor("CTXD", [128, 2, L], BF, kind="ExternalOutput").ap()
                VD = nc.dram_tensor("VD", [128, NB, GH, 65], BF, kind="ExternalOutput").ap()
                for i in range(2):
                    nc.sync.dma_start(out=QTD[:, i], in_=qt_sb[i][:])
                    nc.sync.dma_start(out=KTD[:, i], in_=kt_sb[i][:])
                    nc.sync.dma_start(out=CTXD[:, i], in_=ctx_sb[i][:])
                nc.sync.dma_start(out=VD[:], in_=v_sb[:])

    nc.compile()
    return nc


def kernel(Q, K, V, mask, Wq_w, Wq_b, Wk_w, Wk_b, Wv_w, Wv_b, fc_w, fc_b):
    global LAST_EXEC_NS
    Q = np.asarray(Q, np.float32)
    K = np.asarray(K, np.float32)
    V = np.asarray(V, np.float32)
    mask2d = np.asarray(mask).reshape(L, L).astype(bool)
    Wq_w = np.asarray(Wq_w, np.float32)
    Wq_b = np.asarray(Wq_b, np.float32)
    Wk_w = np.asarray(Wk_w, np.float32)
    Wk_b = np.asarray(Wk_b, np.float32)
    Wv_w = np.asarray(Wv_w, np.float32)
    Wv_b = np.asarray(Wv_b, np.float32)
    fc_w = np.asarray(fc_w, np.float32)
    fc_b = np.asarray(fc_b, np.float32)

    cls = _classify(mask2d)
    zv = not Wv_b.any()
    zf = not fc_b.any()
    key = (cls.tobytes(), zv, zf)
    if key not in _CACHE:
        _CACHE[key] = _build(cls, zv, zf)
    nc = _CACHE[key]

    bf = ml_dtypes.bfloat16
    mixed = _mixed_list(cls)
    if mixed:
        mchunks = np.stack([
            np.ascontiguousarray(mask2d[qb * 128:(qb + 1) * 128, kb * 128:(kb + 1) * 128].T)
            for qb, kb in mixed
        ]).astype(np.uint8)
    else:
        mchunks = np.zeros((1, 128, 128), np.uint8)
    ones_row = np.ones((1, L), np.float32)

    xt = {}
    for b in range(2):
        xt[("Q", b)] = np.ascontiguousarray(Q[b].T).astype(bf)
        xt[("K", b)] = np.ascontiguousarray(K[b].T).astype(bf)
        xt[("V", b)] = np.concatenate([np.ascontiguousarray(V[b].T), ones_row], 0).astype(bf)

    def wlayout(wT):
        # [D, DG] -> [partition 128, kt 8, DG] contiguous
        return np.ascontiguousarray(
            wT.reshape(8, 128, DG).transpose(1, 0, 2)
        ).astype(bf)

    in_maps = []
    for c in range(8):
        b, g = c // 4, c % 4
        sl = slice(g * DG, (g + 1) * DG)
        fc_last = fc_b[None, :] if g == 0 else np.zeros((1, D), np.float32)
        in_maps.append({
            "XTQ": xt[("Q", b)],
            "XTK": xt[("K", b)],
            "XTV": xt[("V", b)],
            "WQT": wlayout(Wq_w[sl, :].T),
            "WKT": wlayout(Wk_w[sl, :].T),
            "WVT": wlayout(Wv_w[sl, :].T),
            "VROW": np.ascontiguousarray(Wv_b[sl][None, :]).astype(bf),
            "BQ": np.ascontiguousarray(
                Wq_b[sl].reshape(2, 128).T.reshape(128, 2, 1)),
            "BK": np.ascontiguousarray(
                Wk_b[sl].reshape(2, 128).T.reshape(128, 2, 1)),
            "FCT": np.ascontiguousarray(
                fc_w[:, sl].T.reshape(2, 128, D).transpose(1, 0, 2)
            ).astype(bf),
            "FCB": np.ascontiguousarray(fc_last).astype(bf),
            "MCHUNKS": mchunks,
        })

    if TRACE:
        _install_ntff_hook()
    res = bass_utils.run_bass_kernel_spmd(
        nc, in_maps, core_ids=list(range(8)),
        trace=TRACE, trace_cores=list(range(8)) if TRACE else None,
    )
    LAST_EXEC_NS = res.exec_time_ns

    out = np.zeros((2, L, D), np.float32)
    for c in range(8):
        out[c // 4] += np.asarray(res.results[c]["Y"]).astype(np.float32)
    return out


# revision 32
# speedup vs baseline: 1.0362x; 1.0362x over previous
"""Multi-head attention (B=2, L=2048, D=1024, H=16) on 8 trn2 NeuronCores.

Sharding: core c = (batch b = c // 4, head-group g = c % 4); each group owns 4
heads (256 dims). Q/K/V projections are column-parallel per group, attention is
fully local per (batch, head), fc is row-parallel with the 4 group partials of
each batch summed on the host.

Per-core dataflow (matmul operands bf16, PSUM accumulation fp32):
  qT,kT [256, L] = W @ x.T          (host supplies x.T and W.T slices)
  v     [L, 256] (+ ones column)    (bias via augmented contraction row)

Attention runs in (key-block, head-pair) units. The two heads of a pair live
in partitions 0-63 / 64-127 of one kt/qt tile, so their score matmuls are
64x128 row-tiled (tile_position (0,0) and (64,0)) and execute CONCURRENTLY on
the PE when emitted back to back into different PSUM banks of one shared
[128, 1024] tile. One ScalarE exp covers both heads; masked diagonal columns
are computed as real scores (finite) and zeroed by gpsimd mask multiplies, so
the merged exp never sees stale PSUM. PV keeps the augmented-v form (M=65,
row 64 = softmax denominator). Normalization: DVE reciprocal straight from
PSUM row 64 -> gpsimd partition_broadcast -> DVE multiply into ctx.

Critical path: only the kb0-3 K quarter and the span-0 Q quarter are
projected before attention starts, so the first exp fires ~20us in (vs ~41us
when K/Q are fully projected first). All other projections (K rest, V, Q
half 1) and the fc units run as cost-paced PE fillers inside the exp-chain
shadow. DMA triggers are split across the sync/scalar/gpsimd queues so the
critical K and Q tiles race down independent queues at t=0.
"""

import numpy as np
import ml_dtypes

import concourse.bass as bass
import concourse.mybir as mybir
import concourse.tile as tile
from concourse import bacc, bass_utils

L = 2048
D = 1024
DK = 64
GH = 4            # heads per core
DG = 256          # dims per core
NB = L // 128     # 16 key/query blocks
NSPAN = L // 512  # 4 query spans
F32 = mybir.dt.float32
BF = mybir.dt.bfloat16
U8 = mybir.dt.uint8

_CACHE: dict = {}
LAST_EXEC_NS = None
TRACE = False


def _install_ntff_hook():
    """Register the axon NTFF profiling hook that this image's antenv lacks."""
    import contextlib
    import ctypes
    import sys
    import types

    try:
        from antenv.axon_hooks import get_axon_ntff_profile_hook  # noqa: F401
        return
    except ImportError:
        pass
    import antenv

    mod = types.ModuleType("antenv.axon_hooks")
    state = {"hook": None}
    mod.set_axon_ntff_profile_hook = lambda h: state.__setitem__("hook", h)
    mod.get_axon_ntff_profile_hook = lambda: state["hook"]
    sys.modules["antenv.axon_hooks"] = mod
    antenv.axon_hooks = mod

    so_path = "/opt/axon/libaxon_pjrt.so"
    lib = ctypes.CDLL(so_path)
    if not hasattr(lib, "axon_start_nrt_profile"):
        return
    lib.axon_start_nrt_profile.argtypes = [
        ctypes.POINTER(ctypes.c_int64),
        ctypes.c_size_t,
    ]
    lib.axon_start_nrt_profile.restype = ctypes.c_int64
    lib.axon_stop_nrt_profile.argtypes = [ctypes.c_char_p]
    lib.axon_stop_nrt_profile.restype = ctypes.c_int64

    @contextlib.contextmanager
    def _hook(output_dir, device_ids):
        import jax

        jax.devices()
        if device_ids:
            ids = (ctypes.c_int64 * len(device_ids))(*device_ids)
            rc = lib.axon_start_nrt_profile(ids, len(device_ids))
        else:
            rc = lib.axon_start_nrt_profile(None, 0)
        if rc != 0:
            raise RuntimeError(f"axon_start_nrt_profile rc={rc}")
        try:
            yield
        finally:
            n = lib.axon_stop_nrt_profile(str(output_dir).encode())
            print(f"profile: {n} file(s) written to {output_dir}", file=sys.stderr)

    state["hook"] = _hook


def _classify(mask2d: np.ndarray) -> np.ndarray:
    """cls[qb, kb]: 0 = all masked (dead), 1 = all unmasked (pure), 2 = mixed."""
    m = mask2d.astype(np.uint8).reshape(NB, 128, NB, 128)
    s = m.sum(axis=(1, 3))
    cls = np.full((NB, NB), 2, np.int8)
    cls[s == 0] = 0
    cls[s == 128 * 128] = 1
    return cls


def _mixed_list(cls):
    return [(qb, kb) for qb in range(NB) for kb in range(NB) if cls[qb, kb] == 2]


DEBUG_DUMP = False


def _build(cls: np.ndarray, zv: bool = False, zf: bool = False):
    nc = bacc.Bacc("TRN2", target_bir_lowering=False, debug=False, num_devices=8)
    XTQ = nc.dram_tensor("XTQ", [D, L], BF, kind="ExternalInput").ap()
    XTK = nc.dram_tensor("XTK", [D, L], BF, kind="ExternalInput").ap()
    XTV = nc.dram_tensor("XTV", [D + 1, L], BF, kind="ExternalInput").ap()
    WQT = nc.dram_tensor("WQT", [128, 8, DG], BF, kind="ExternalInput").ap()
    WKT = nc.dram_tensor("WKT", [128, 8, DG], BF, kind="ExternalInput").ap()
    WVT = nc.dram_tensor("WVT", [128, 8, DG], BF, kind="ExternalInput").ap()
    VROW = nc.dram_tensor("VROW", [1, DG], BF, kind="ExternalInput").ap()
    BQ = nc.dram_tensor("BQ", [128, 2, 1], F32, kind="ExternalInput").ap()
    BK = nc.dram_tensor("BK", [128, 2, 1], F32, kind="ExternalInput").ap()
    FCT = nc.dram_tensor("FCT", [128, 2, D], BF, kind="ExternalInput").ap()
    FCB = nc.dram_tensor("FCB", [1, D], BF, kind="ExternalInput").ap()
    mixed = _mixed_list(cls)
    nmix = max(1, len(mixed))
    MCHUNKS = nc.dram_tensor("MCHUNKS", [nmix, 128, 128], U8, kind="ExternalInput").ap()
    Y = nc.dram_tensor("Y", [L, D], BF, kind="ExternalOutput").ap()
    PTD = DEND = None
    if DEBUG_DUMP:
        PTD = nc.dram_tensor("PTD", [128, 2, 1024], BF, kind="ExternalOutput").ap()
        DEND = nc.dram_tensor("DEND", [2, 512], F32, kind="ExternalOutput").ap()

    # per-span live key blocks (shared by all heads; mask broadcasts)
    span_kbs = []
    for s in range(NSPAN):
        kbs = [kb for kb in range(NB) if any(cls[4 * s + j, kb] for j in range(4))]
        assert kbs, f"query span {s} has no unmasked keys"
        span_kbs.append(kbs)

    Exp = mybir.ActivationFunctionType.Exp

    with tile.TileContext(nc) as tc:
        with (
            tc.tile_pool(name="w", bufs=1) as wp,
            tc.tile_pool(name="xs", bufs=24) as xp,
            tc.tile_pool(name="keep", bufs=1) as kp,
            tc.tile_pool(name="ptp", bufs=5) as ptp,
            tc.tile_pool(name="sm", bufs=2) as smp,
            tc.tile_pool(name="ev", bufs=4) as evp,
            tc.tile_pool(name="pout", bufs=2, space="PSUM") as pout,
            tc.tile_pool(name="psc", bufs=2, space="PSUM") as psc,
            tc.tile_pool(name="pov", bufs=2, space="PSUM") as pov,
        ):
            # ---------------- persistent activations ----------------
            qt_sb = [kp.tile([128, L], BF, tag=f"qt{i}", name=f"qt{i}") for i in range(2)]
            kt_sb = [kp.tile([128, L], BF, tag=f"kt{i}", name=f"kt{i}") for i in range(2)]
            ctx_sb = [kp.tile([128, L], BF, tag=f"ctx{i}", name=f"ctx{i}") for i in range(2)]
            v_sb = kp.tile([128, NB, GH, 65], BF, tag="vsb")

            # ---------------- weights ----------------
            wkt0 = wp.tile([128, DG], BF, tag="wkt0", name="wkt0")
            wkt17 = wp.tile([128, 7, DG], BF, tag="wkt17", name="wkt17")
            wqt = wp.tile([128, 8, DG], BF, tag="wqt")
            wvt = wp.tile([128, 8, DG], BF, tag="wvt")
            vrow = wp.tile([1, DG], BF, tag="vrow")
            fct = wp.tile([128, 2, D], BF, tag="fct")
            fcb = wp.tile([1, D], BF, tag="fcb")
            bq = wp.tile([128, 2, 1], F32, tag="bq")
            bk = wp.tile([128, 2, 1], F32, tag="bk")

            # ---------------- DMA staging ----------------
            # everything on the sync queue (compute-engine DMA triggers
            # measured 2-4x slower), strictly in consumption order: K weights
            # first (a chain stalls on wkt17 if it queues behind the x
            # stream), then the critical K/Q quarter tiles, then the rest.
            nc.sync.dma_start(out=wkt0[:], in_=WKT[:, 0])
            nc.sync.dma_start(out=wkt17[:], in_=WKT[:, 1:8])
            nc.sync.dma_start(out=wqt[:], in_=WQT[:])

            def load_half(src, half, name, cols=None):
                # one tile + DMA per k-tile; cols=(lo, hi) loads only that
                # column slice (the other slice is DMAed later). All 48 x
                # tiles are DMA-triggered up front, so every tile gets its
                # own slot (bufs=48) - slot rotation would clobber tiles
                # whose reader chains are emitted later as fillers.
                ts = []
                lo, hi = cols or (0, 1024)
                for kt in range(8):
                    t = xp.tile([128, 1024], BF, tag="xt", bufs=48,
                                name=f"{name}{half}k{kt}")
                    nc.sync.dma_start(
                        out=t[:, lo:hi],
                        in_=src[kt * 128:(kt + 1) * 128,
                                half * 1024 + lo:half * 1024 + hi],
                    )
                    ts.append(t)
                return ts

            def load_cols(ts, src, half, lo, hi):
                for kt in range(8):
                    nc.sync.dma_start(
                        out=ts[kt][:, lo:hi],
                        in_=src[kt * 128:(kt + 1) * 128,
                                half * 1024 + lo:half * 1024 + hi],
                    )

            xtk0_ts = load_half(XTK, 0, "xtk", cols=(0, 512))
            xtq0_ts = load_half(XTQ, 0, "xtq", cols=(0, 512))
            nc.sync.dma_start(out=bk[:], in_=BK[:])
            nc.sync.dma_start(out=bq[:], in_=BQ[:])

            # 0/1 chunks for mixed mask blocks (needed by span 0 already)
            m01_idx = {qk: i for i, qk in enumerate(mixed)}
            m01_all = wp.tile([128, nmix, 128], BF, tag="m01")
            mstage = wp.tile([128, nmix, 128], U8, tag="mstage")
            if mixed:
                nc.sync.dma_start(out=mstage[:], in_=MCHUNKS.rearrange("n p c -> p n c"))
                nc.gpsimd.tensor_copy(m01_all[:], mstage[:])

            # rest of the x stream, in consumption order: V half 0 before the
            # K/Q second halves (span-0 PVs need v0-3 before span 1 needs
            # kt/qt quarter 1)
            nc.sync.dma_start(out=wvt[:], in_=WVT[:])
            nc.sync.dma_start(out=vrow[:], in_=VROW[:])
            xr = xp.tile([1, L], BF, tag="xtr", bufs=1, name="xr")
            nc.sync.dma_start(out=xr[:], in_=XTV[D:D + 1])
            xtv_ts = load_half(XTV, 0, "xtv")
            load_cols(xtk0_ts, XTK, 0, 512, 1024)
            load_cols(xtq0_ts, XTQ, 0, 512, 1024)
            xtk1_ts = load_half(XTK, 1, "xtk")
            xtv_ts += load_half(XTV, 1, "xtv")
            nc.sync.dma_start(out=fct[:], in_=FCT[:])
            nc.sync.dma_start(out=fcb[:], in_=FCB[:])
            xtq1_ts = load_half(XTQ, 1, "xtq")
            xtk_halves = [xtk0_ts, xtk1_ts]
            xtq_halves = [xtq0_ts, xtq1_ts]

            # ---------------- constants (off the DMA-trigger path) --------
            ctx1 = wp.tile([1, L], BF, tag="ctx1")
            nc.vector.memset(ctx1[:], 1.0)
            nc.vector.memset(v_sb[:, :, :, 64:65], 1.0)

            wk_slice = lambda kt, mch: (
                wkt0[:, mch * 128:(mch + 1) * 128] if kt == 0
                else wkt17[:, kt - 1, mch * 128:(mch + 1) * 128]
            )
            wq_slice = lambda kt, mch: wqt[:, kt, mch * 128:(mch + 1) * 128]

            def proj_chain(q, mch, halves, wsl, bias, dst):
                # one L-quarter (512 cols) x one 128-row output chunk
                ts = halves[q // 2]
                c0 = (q % 2) * 512
                p = pout.tile([128, 512], F32, tag="out", name=f"pp{q}_{mch}")
                for kt in range(8):
                    nc.tensor.matmul(
                        p[:], wsl(kt, mch), ts[kt][:, c0:c0 + 512],
                        start=(kt == 0), stop=(kt == 7),
                    )
                nc.vector.tensor_scalar_add(
                    dst[mch][:, q * 512:(q + 1) * 512], p[:], bias[:, mch]
                )

            v_done = [False] * NB

            def v_chunk(lb):
                if v_done[lb]:
                    return
                v_done[lb] = True
                half, lbl = lb // 8, lb % 8
                p = pout.tile([128, DG], F32, tag="out", name=f"pv{lb}")
                for kt in range(8):
                    nc.tensor.matmul(
                        p[:],
                        xtv_ts[half * 8 + kt][:, lbl * 128:(lbl + 1) * 128],
                        wvt[:, kt],
                        start=(kt == 0),
                        stop=(zv and kt == 7),
                    )
                if not zv:
                    nc.tensor.matmul(
                        p[:], xr[:, lb * 128:(lb + 1) * 128], vrow[:],
                        start=False, stop=True,
                    )
                nc.vector.tensor_copy(
                    v_sb[:, lb, :, 0:64], p[:].rearrange("p (h d) -> p h d", h=GH)
                )

            drain_fc = [False]

            def fc_unit(lb, nh):
                ys = evp.tile([128, 512], BF, tag="ys", name=f"ys{lb}_{nh}")
                pool = psc if drain_fc[0] else pout
                yp = pool.tile([128, 512], F32, tag="sc" if pool is psc else "out",
                               name=f"yp{lb}_{nh}")
                nc.tensor.matmul(
                    yp[:], ctx_sb[0][:, lb * 128:(lb + 1) * 128],
                    fct[:, 0, nh * 512:(nh + 1) * 512], start=True, stop=False,
                    skip_group_check=True,
                )
                nc.tensor.matmul(
                    yp[:], ctx_sb[1][:, lb * 128:(lb + 1) * 128],
                    fct[:, 1, nh * 512:(nh + 1) * 512], start=False, stop=zf,
                    skip_group_check=True,
                )
                if not zf:
                    nc.tensor.matmul(
                        yp[:], ctx1[:, lb * 128:(lb + 1) * 128],
                        fcb[:, nh * 512:(nh + 1) * 512], start=False, stop=True,
                        skip_group_check=True,
                    )
                nc.vector.tensor_copy(ys[:], yp[:])
                nc.sync.dma_start(
                    out=Y[lb * 128:(lb + 1) * 128, nh * 512:(nh + 1) * 512],
                    in_=ys[:],
                )

            # ---------------- filler machinery ----------------
            # (cost_ns, key, closure); popped against a per-unit budget so
            # the PE always has work inside the exp-chain shadow without
            # starving the score stream. force_keys() hard-pops (in order)
            # until named producers have been EMITTED - emission order is
            # dependency order for the tile framework.
            fillers = []
            urgent = []
            budget = [0.0]
            done_keys = set()

            def _pop_filler():
                c, k, f = fillers.pop(0)
                budget[0] = max(budget[0] - c, -1500.0)
                if k is not None:
                    done_keys.add(k)
                f()

            def pace(amount):
                budget[0] += amount
                while fillers and fillers[0][0] <= budget[0]:
                    _pop_filler()

            def force_keys(keys):
                keys = [k for k in keys if k not in done_keys]
                while keys and fillers:
                    _pop_filler()
                    keys = [k for k in keys if k not in done_keys]
                assert not keys, f"missing filler producers {keys}"

            def flush_urgent():
                while urgent:
                    urgent.pop(0)()

            # ---------------- attention ----------------
            def first_col(kb, s):
                if kb == span_kbs[s][0]:
                    return 0
                j0 = next(j for j in range(4) if cls[4 * s + j, kb])
                return j0 * 128

            def norm_prep(st):
                # NOTE: reciprocal_approx_fast mis-reads a PSUM input with a
                # nonzero partition base (it reads partition 0) - copy the
                # denominator row to SBUF partition 0 first.
                outp = st["outp"]
                srow = smp.tile([1, 512], F32, tag="srow", name="srow")
                nc.vector.tensor_copy(srow[:], outp[64:65, :])
                rc = smp.tile([1, 512], F32, tag="rc", name="rc")
                nc.vector.reciprocal_approx_fast(rc[:], srow[:])
                bcs = smp.tile([64, 512], F32, tag="bcs", name="bcs")
                nc.gpsimd.partition_broadcast(bcs[:], rc[:])
                st["bcs"] = bcs
                if DEBUG_DUMP and st["s"] == 0 and st["h"] < 2:
                    nc.sync.dma_start(out=DEND[st["h"]:st["h"] + 1], in_=bcs[0:1, :])

            def norm_apply(st, on_done=None):
                h, s, outp, bcs = st["h"], st["s"], st["outp"], st["bcs"]
                hi, ho = h // 2, (h % 2) * 64
                nc.vector.tensor_mul(
                    ctx_sb[hi][ho:ho + 64, s * 512:(s + 1) * 512],
                    outp[0:64, :],
                    bcs[:],
                )
                if on_done is not None:
                    on_done()

            pending_pv = []

            def run_pv(n_keep):
                while len(pending_pv) > n_keep:
                    pending_pv.pop(0)()

            def emit_span(s):
                kbs = span_kbs[s]
                # deep PV lag: the next pair's first PV carries a WAR wait on
                # the previous pair's norm chain (~3.5us); keep scores and
                # fillers flowing ahead of it
                lag = 3
                for hp in range(2):
                    # the previous pair's norms MUST be emitted before this
                    # pair's PVs reuse their PSUM slots (emission order is
                    # dependency order)
                    flush_urgent()
                    outps = [
                        pov.tile([65, 512], F32, tag="ov", name=f"ov{2 * hp + i}_{s}")
                        for i in range(2)
                    ]
                    for ki, kb in enumerate(kbs):
                        c0 = first_col(kb, s)
                        scp = psc.tile([128, 1024], F32, tag="sc", name="scp")
                        # paired 64-row scores: heads 2hp / 2hp+1 in row tiles
                        # (0,0) and (64,0), concurrent on the PE
                        for hh in range(2):
                            nc.tensor.matmul(
                                scp[:, hh * 512 + c0:(hh + 1) * 512],
                                kt_sb[hp][hh * 64:(hh + 1) * 64,
                                          kb * 128:(kb + 1) * 128],
                                qt_sb[hp][hh * 64:(hh + 1) * 64,
                                          s * 512 + c0:(s + 1) * 512],
                                start=True, stop=True,
                            )
                        pt = ptp.tile([128, 1024], BF, tag="pt", name="pt")
                        # one exp for both heads; [512:512+c0] is stale PSUM
                        # whose exp lands in a pt strip nothing ever reads
                        nc.scalar.activation(pt[:, c0:], scp[:, c0:], Exp, scale=0.125)
                        for j in range(c0 // 128, 4):
                            qb = 4 * s + j
                            c = cls[qb, kb]
                            if c == 1:
                                continue
                            for hh in range(2):
                                sl = pt[:, hh * 512 + j * 128:hh * 512 + (j + 1) * 128]
                                if c == 0:
                                    nc.vector.memset(sl, 0.0)
                                else:
                                    nc.vector.tensor_mul(
                                        sl, sl, m01_all[:, m01_idx[(qb, kb)], :]
                                    )

                        def pv(kb=kb, c0=c0, pt=pt, outps=outps, hp=hp,
                               first=(ki == 0), last=(ki == len(kbs) - 1)):
                            v_chunk(kb)  # backstop; usually a no-op
                            for hh in range(2):
                                nc.tensor.matmul(
                                    outps[hh][:, c0:],
                                    v_sb[:, kb, 2 * hp + hh, :],
                                    pt[:, hh * 512 + c0:(hh + 1) * 512],
                                    start=first, stop=last,
                                    skip_group_check=True,
                                )

                        if DEBUG_DUMP and s == 0 and hp == 0 and kb <= 1:
                            nc.sync.dma_start(out=PTD[:, kb], in_=pt[:])
                        pending_pv.append(pv)
                        run_pv(lag)
                        pace(560)
                    run_pv(0)
                    sts = []
                    for hh in range(2):
                        h = 2 * hp + hh
                        st = {"h": h, "s": s, "outp": outps[hh]}
                        sts.append(st)
                        urgent.append(lambda st=st: norm_prep(st))
                    for hh in range(2):
                        st = sts[hh]
                        on_done = None
                        if hp == 1 and hh == 1:
                            def on_done(s=s):
                                # span ctx complete -> fc is legal; front of
                                # the queue so fc drains early (small units,
                                # latency-critical at the end of the kernel)
                                for i, lb in enumerate(range(4 * s, 4 * s + 4)):
                                    for nh in range(2):
                                        fillers.insert(
                                            2 * i + nh,
                                            (430, None,
                                             lambda lb=lb, nh=nh: fc_unit(lb, nh)))
                        urgent.append(lambda st=st, od=on_done: norm_apply(st, od))

            # critical-path projections: K quarter 0 (kb 0-3) + Q span 0
            for mch in range(2):
                proj_chain(0, mch, xtk_halves, wk_slice, bk, kt_sb)
            for mch in range(2):
                proj_chain(0, mch, xtq_halves, wq_slice, bq, qt_sb)

            # everything else becomes fillers, ordered by first use
            CH = 1700

            def add_k(q):
                for mch in range(2):
                    fillers.append((CH, ("k", q, mch), lambda q=q, mch=mch: proj_chain(
                        q, mch, xtk_halves, wk_slice, bk, kt_sb)))

            def add_q(q):
                for mch in range(2):
                    fillers.append((CH, ("q", q, mch), lambda q=q, mch=mch: proj_chain(
                        q, mch, xtq_halves, wq_slice, bq, qt_sb)))

            def add_v(lbs):
                for lb in lbs:
                    fillers.append((870, None, lambda lb=lb: v_chunk(lb)))

            add_k(1)
            add_q(1)
            add_v(range(4))
            add_k(2)
            add_q(2)
            add_k(3)
            add_q(3)
            add_v(range(4, 16))

            def span_need(s):
                ks = {("k", q, mch) for q in range(1, s + 1) for mch in range(2)}
                return ks | {("q", s, mch) for mch in range(2)}

            for s in range(NSPAN):
                if s:
                    force_keys(span_need(s))
                emit_span(s)
            run_pv(0)
            flush_urgent()
            drain_fc[0] = True
            while fillers:
                fillers.pop(0)[2]()

            if DEBUG_DUMP:
                QTD = nc.dram_tensor("QTD", [128, 2, L], BF, kind="ExternalOutput").ap()
                KTD = nc.dram_tensor("KTD", [128, 2, L], BF, kind="ExternalOutput").ap()
                CTXD = nc.dram_tensor("CTXD", [128, 2, L], BF, kind="ExternalOutput").ap()
                VD = nc.dram_tensor("VD", [128, NB, GH, 65], BF, kind="ExternalOutput").ap()
                for i in range(2):
                    nc.sync.dma_start(out=QTD[:, i], in_=qt_sb[i][:])
                    nc.sync.dma_start(out=KTD[:, i], in_=kt_sb[i][:])
                    nc.sync.dma_start(out=CTXD[:, i], in_=ctx_sb[i][:])
                nc.sync.dma_start(out=VD[:], in_=v_sb[:])

    nc.compile()
    return nc


def kernel(Q, K, V, mask, Wq_w, Wq_b, Wk_w, Wk_b, Wv_w, Wv_b, fc_w, fc_b):
    global LAST_EXEC_NS
    Q = np.asarray(Q, np.float32)
    K = np.asarray(K, np.float32)
    V = np.asarray(V, np.float32)
    mask2d = np.asarray(mask).reshape(L, L).astype(bool)
    Wq_w = np.asarray(Wq_w, np.float32)
    Wq_b = np.asarray(Wq_b, np.float32)
    Wk_w = np.asarray(Wk_w, np.float32)
    Wk_b = np.asarray(Wk_b, np.float32)
    Wv_w = np.asarray(Wv_w, np.float32)
    Wv_b = np.asarray(Wv_b, np.float32)
    fc_w = np.asarray(fc_w, np.float32)
    fc_b = np.asarray(fc_b, np.float32)

    cls = _classify(mask2d)
    zv = not Wv_b.any()
    zf = not fc_b.any()
    key = (cls.tobytes(), zv, zf)
    if key not in _CACHE:
        _CACHE[key] = _build(cls, zv, zf)
    nc = _CACHE[key]

    bf = ml_dtypes.bfloat16
    mixed = _mixed_list(cls)
    if mixed:
        mchunks = np.stack([
            np.ascontiguousarray(mask2d[qb * 128:(qb + 1) * 128, kb * 128:(kb + 1) * 128].T)
            for qb, kb in mixed
        ]).astype(np.uint8)
    else:
        mchunks = np.zeros((1, 128, 128), np.uint8)
    ones_row = np.ones((1, L), np.float32)

    xt = {}
    for b in range(2):
        xt[("Q", b)] = np.ascontiguousarray(Q[b].T).astype(bf)
        xt[("K", b)] = np.ascontiguousarray(K[b].T).astype(bf)
        xt[("V", b)] = np.concatenate([np.ascontiguousarray(V[b].T), ones_row], 0).astype(bf)

    def wlayout(wT):
        # [D, DG] -> [partition 128, kt 8, DG] contiguous
        return np.ascontiguousarray(
            wT.reshape(8, 128, DG).transpose(1, 0, 2)
        ).astype(bf)

    in_maps = []
    for c in range(8):
        b, g = c // 4, c % 4
        sl = slice(g * DG, (g + 1) * DG)
        fc_last = fc_b[None, :] if g == 0 else np.zeros((1, D), np.float32)
        in_maps.append({
            "XTQ": xt[("Q", b)],
            "XTK": xt[("K", b)],
            "XTV": xt[("V", b)],
            "WQT": wlayout(Wq_w[sl, :].T),
            "WKT": wlayout(Wk_w[sl, :].T),
            "WVT": wlayout(Wv_w[sl, :].T),
            "VROW": np.ascontiguousarray(Wv_b[sl][None, :]).astype(bf),
            "BQ": np.ascontiguousarray(
                Wq_b[sl].reshape(2, 128).T.reshape(128, 2, 1)),
            "BK": np.ascontiguousarray(
                Wk_b[sl].reshape(2, 128).T.reshape(128, 2, 1)),
            "FCT": np.ascontiguousarray(
                fc_w[:, sl].T.reshape(2, 128, D).transpose(1, 0, 2)
            ).astype(bf),
            "FCB": np.ascontiguousarray(fc_last).astype(bf),
            "MCHUNKS": mchunks,
        })

    if TRACE:
        _install_ntff_hook()
    res = bass_utils.run_bass_kernel_spmd(
        nc, in_maps, core_ids=list(range(8)),
        trace=TRACE, trace_cores=list(range(8)) if TRACE else None,
    )
    LAST_EXEC_NS = res.exec_time_ns

    out = np.zeros((2, L, D), np.float32)
    for c in range(8):
        out[c // 4] += np.asarray(res.results[c]["Y"]).astype(np.float32)
    return out
